# revision 50
# baseline (speedup 1.0000x reference)
# Multi-head attention (b=2, n=2048, d_model=1024, 16 heads) on 8 NeuronCores.
#
# Sharding: core c = (batch b, head-group g) with b = c//4, g = c%4.
# Each core handles 1 batch element and 4 heads (256 channels), computing a
# partial output projection; the host sums the 4 group-partials per batch and
# adds b_O.
#
# Device layout (everything oriented so no transposes are needed):
#   xT   [D, N]      = x[b].T (bf16)             rhs of Q/K proj, lhsT of V
#   Q.T/K.T [2][128, N]: two heads per 128-row tile (head 2p at rows 0:64,
#        head 2p+1 at rows 64:128).  Scores matmuls for the two heads of a
#        pair contract K=64 on DISJOINT PE row-groups (tile_position rows 0
#        vs 64, auto-derived from the lhsT base partition), so issuing them
#        back-to-back runs them CONCURRENTLY -> 2x scores throughput, and
#        each head's kt LDWEIGHTS hides behind the other head's matmul.
#   V    [N, CH] natural layout (+b_v), stored per-head with an appended
#        ones column: lhsT [m, 65] so the O-matmul's PSUM row 64 accumulates
#        the softmax denominators for free.
#   S.T  [128(m-slice), 1024] fp32 PSUM = both heads' scores for one
#        (n-window, m-slice): head A cols 0:512, head B cols 512:1024.
#        ONE ScalarE activation (exp, scale 1/8 folded) covers the pair.
#   O.T+sums [65, 512] per head = [V_h | 1].T @ E.T  (accumulate over m)
#   Y.T  [D, N] bf16 = woT.T @ (O.T * recip(sums)), per-chunk fillers.
#
# Matmul operands are bf16 (fp32 PSUM accumulation).

import ml_dtypes
import numpy as np

import concourse.bass as bass
import concourse.bacc as bacc
import concourse.tile as tile
from concourse import mybir
from concourse.bass_utils import run_bass_kernel_spmd

D = 1024  # d_model
N = 2048  # sequence length
B = 2  # batch
NHEADS = 16
DK = 64
NCORES = 8
GROUPS = 4  # head-groups across cores
HPG = NHEADS // GROUPS  # 4 heads per group
CH = HPG * DK  # 256 channels per group
KT = D // 128  # 8 contraction tiles for the projections
MS = N // 128  # 16 m-slices (key dim)
NW = 512  # n-window width for the attention phase
NWIN = N // NW  # 4 windows
NPAIR = HPG // 2  # 2 head pairs per core (= CH//128 tiles)

F32 = mybir.dt.float32
BF16 = mybir.dt.bfloat16


def _build_bass():
    nc = bacc.Bacc()

    xT_d = nc.dram_tensor("xT", [D, N], BF16, kind="ExternalInput")
    wqT_d = nc.dram_tensor("wqT", [D, CH], BF16, kind="ExternalInput")
    wkT_d = nc.dram_tensor("wkT", [D, CH], BF16, kind="ExternalInput")
    wvT_d = nc.dram_tensor("wvT", [D, CH], BF16, kind="ExternalInput")
    woT_d = nc.dram_tensor("woT", [CH, D], BF16, kind="ExternalInput")
    bq_d = nc.dram_tensor("bq", [CH], F32, kind="ExternalInput")
    bk_d = nc.dram_tensor("bk", [CH], F32, kind="ExternalInput")
    bv_d = nc.dram_tensor("bv", [CH], F32, kind="ExternalInput")
    vones_d = nc.dram_tensor("vones", [128, HPG], BF16, kind="ExternalInput")
    yT_d = nc.dram_tensor("yT", [D, N], BF16, kind="ExternalOutput")

    with tile.TileContext(nc) as tc:
        with (
            tc.tile_pool(name="persist", bufs=1) as persist,
            tc.tile_pool(name="ph1", bufs=1) as ph1,
            tc.tile_pool(name="et_pool", bufs=4) as et_pool,
            tc.tile_pool(name="osb_pool", bufs=2) as osb_pool,
            tc.tile_pool(name="small", bufs=2) as small,
            tc.tile_pool(name="aux_ps", bufs=2, space="PSUM") as aux_ps,
            tc.tile_pool(name="st_ps", bufs=2, space="PSUM") as st_pool,
            tc.tile_pool(name="ot_ps", bufs=1, space="PSUM") as ot_pool,
        ):
            # ---- input loads.  x is loaded COLUMN-MAJOR in [128, 512]
            # ---- blocks spread over the three DMA-capable queues: the
            # ---- first attention iterations only touch column-block 0, so
            # ---- compute starts ~12us in while the rest of x streams.
            xt = [ph1.tile([128, N], BF16, tag=f"xt{k}", name=f"xt{k}") for k in range(KT)]
            wq, wk, wv = [], [], []
            for wname, dram, lst in (("wk", wkT_d, wk), ("wq", wqT_d, wq), ("wv", wvT_d, wv)):
                for k in range(KT):
                    lst.append(ph1.tile([128, CH], BF16, tag=f"{wname}{k}", name=f"{wname}{k}"))
            # tier 0 on gpsimd: tiny biases first, then wq
            bq_t, bk_t = [], []
            for bname, dram, lst in (("bq", bq_d, bq_t), ("bk", bk_d, bk_t)):
                for cs in range(CH // 128):
                    t = ph1.tile([128, 1], F32, tag=f"{bname}{cs}", name=f"{bname}{cs}")
                    nc.gpsimd.dma_start(out=t, in_=dram[cs * 128 : (cs + 1) * 128])
                    lst.append(t)
            bvb = ph1.tile([128, CH], F32, tag="bvb", name="bvb")
            bv_ap = bv_d[None, :]
            nc.gpsimd.dma_start(
                out=bvb,
                in_=bass.AP(tensor=bv_ap.tensor, offset=bv_ap.offset, ap=[[0, 128]] + list(bv_ap.ap[1:])),
            )
            # persistent all-ones [128, HPG] for the V ones-columns (written
            # per-tile by a cheap DVE copy — a per-emit_v DMA would queue
            # behind the x-block transfers and stall the first AV by ~5us)
            vones_sb = ph1.tile([128, HPG], BF16, tag="vones_sb", name="vones_sb")
            nc.gpsimd.dma_start(out=vones_sb, in_=vones_d[:, :])
            for k in range(KT):
                nc.gpsimd.dma_start(out=wq[k], in_=wqT_d[k * 128 : (k + 1) * 128, :])
            # tier 1 on sync/scalar: x column-block 0 interleaved with wk,
            # then wv (needed by the prelude V chains).  The scalar queue
            # carries ONLY this tier: it must drain before the first
            # ACTIVATE (~13us) or exp dispatch stalls behind descriptor-gen.
            qs = [nc.sync, nc.scalar]
            for k in range(KT):
                qs[k % 2].dma_start(out=xt[k][:, 0:512], in_=xT_d[k * 128 : (k + 1) * 128, 0:512])
                qs[k % 2].dma_start(out=wk[k], in_=wkT_d[k * 128 : (k + 1) * 128, :])
            for k in range(KT):
                qs[k % 2].dma_start(out=wv[k], in_=wvT_d[k * 128 : (k + 1) * 128, :])
            # tier 2: remaining x column-blocks, col-major, sync+gpsimd only
            for cb in range(1, 4):
                for k in range(KT):
                    eng = (nc.sync, nc.gpsimd)[(k + cb) % 2]
                    eng.dma_start(
                        out=xt[k][:, cb * 512 : (cb + 1) * 512],
                        in_=xT_d[k * 128 : (k + 1) * 128, cb * 512 : (cb + 1) * 512],
                    )

            # ---- persistent tensors ----
            qt = [persist.tile([128, N], BF16, tag=f"qt{cs}", name=f"qt{cs}") for cs in range(CH // 128)]
            kt = [persist.tile([128, N], BF16, tag=f"kt{cs}", name=f"kt{cs}") for cs in range(CH // 128)]
            v4 = [persist.tile([128, HPG * 65], BF16, tag=f"v4_{ms}", name=f"v4_{ms}") for ms in range(MS)]
            wot = []
            for cs in range(CH // 128):
                t = persist.tile([128, D], BF16, tag=f"wot{cs}", name=f"wot{cs}")
                nc.gpsimd.dma_start(out=t, in_=woT_d[cs * 128 : (cs + 1) * 128, :])
                wot.append(t)

            # ---- filler emitters: one PSUM-chain each, paced into the
            # ---- attention loop so the PE eats them during dependency stalls
            def emit_v(ms):
                ps = aux_ps.tile([128, 512], F32, tag="aux", name="aux_ps_t")
                for k in range(KT):
                    nc.tensor.matmul(
                        ps[:, 0:CH],
                        xt[k][:, ms * 128 : (ms + 1) * 128],
                        wv[k],
                        start=(k == 0),
                        stop=(k == KT - 1),
                    )
                v4v = v4[ms].rearrange("p (h c) -> p h c", c=65)
                nc.vector.tensor_copy(
                    out=v4v[:, :, 64:65],
                    in_=vones_sb.rearrange("p (h c) -> p h c", c=1),
                )
                nc.vector.tensor_add(
                    out=v4v[:, :, 0:64],
                    in0=ps[:, 0:CH].rearrange("p (h c) -> p h c", c=64),
                    in1=bvb.rearrange("p (h c) -> p h c", c=64),
                )

            def emit_qk_chain(isq, cs, n0):
                dst, w, bias = (qt, wq, bq_t) if isq else (kt, wk, bk_t)
                ps = aux_ps.tile([128, 512], F32, tag="aux", name="aux_ps_t")
                for k in range(KT):
                    nc.tensor.matmul(
                        ps,
                        w[k][:, cs * 128 : (cs + 1) * 128],
                        xt[k][:, n0 : n0 + 512],
                        start=(k == 0),
                        stop=(k == KT - 1),
                    )
                nc.vector.tensor_scalar_add(
                    out=dst[cs][:, n0 : n0 + 512], in0=ps, scalar1=bias[cs]
                )

            osb_tiles = {}

            def emit_f(c, msl, tail=False):
                osb = osb_tiles[c]
                yp = aux_ps.tile([128, 512], F32, tag="aux", name="aux_yt_t")
                for cs in range(CH // 128):
                    nc.tensor.matmul(
                        yp,
                        wot[cs][:, msl * 128 : (msl + 1) * 128],
                        osb[cs],
                        start=(cs == 0),
                        stop=(cs == CH // 128 - 1),
                    )
                ysb = small.tile([128, 512], BF16, tag="ysb", name="ysb_t", bufs=4)
                if tail and msl % 2 == 1:
                    # past the last exp, ScalarE is idle: alternating the
                    # PSUM->SBUF copies onto it halves the drain of the
                    # final output-projection chains (Copy is in every
                    # activation table set, so no table reload)
                    nc.scalar.activation(
                        out=ysb, in_=yp, func=mybir.ActivationFunctionType.Copy
                    )
                else:
                    nc.vector.tensor_copy(out=ysb, in_=yp)
                nc.sync.dma_start(
                    out=yT_d[msl * 128 : (msl + 1) * 128, c * NW : (c + 1) * NW],
                    in_=ysb,
                )

            # prelude: only what (chunk 0, pair 0, ms 0..3) needs — all of it
            # depends only on x column-block 0 (plus wk/wq/wv).  The K and Q
            # chains are interleaved per k-tile so each matmul runs as soon
            # as its xt block-0 slice lands instead of serializing on the
            # PE FIFO behind xt[7].
            psK = aux_ps.tile([128, 512], F32, tag="aux", name="psK_t")
            psQ = aux_ps.tile([128, 512], F32, tag="aux", name="psQ_t")
            for k in range(KT):
                nc.tensor.matmul(
                    psK, wk[k][:, 0:128], xt[k][:, 0:512],
                    start=(k == 0), stop=(k == KT - 1),
                )
                nc.tensor.matmul(
                    psQ, wq[k][:, 0:128], xt[k][:, 0:512],
                    start=(k == 0), stop=(k == KT - 1),
                )
            nc.vector.tensor_scalar_add(out=kt[0][:, 0:512], in0=psK, scalar1=bk_t[0])
            nc.vector.tensor_scalar_add(out=qt[0][:, 0:512], in0=psQ, scalar1=bq_t[0])
            for ms in range(3):
                emit_v(ms)

            # deadline-paced fillers: dict iter -> list of thunks, ordered so
            # each lands after its x column-block arrives and before its
            # consumer iteration
            sched = {}

            def add(it, f):
                sched.setdefault(it, []).append(f)

            add(0, lambda: emit_v(3))
            add(1, lambda: emit_v(4))
            add(2, lambda: emit_qk_chain(False, 0, 512))
            add(3, lambda: emit_v(5))
            add(4, lambda: emit_v(6))
            add(5, lambda: emit_qk_chain(False, 0, 1024))
            add(6, lambda: emit_v(7))
            add(7, lambda: emit_v(8))
            add(8, lambda: emit_qk_chain(False, 1, 0))
            add(9, lambda: emit_v(9))
            add(9, lambda: emit_qk_chain(False, 0, 1536))
            add(10, lambda: emit_v(10))
            add(10, lambda: emit_qk_chain(False, 1, 512))
            add(11, lambda: emit_v(11))
            add(12, lambda: emit_v(12))
            add(12, lambda: emit_qk_chain(False, 1, 1024))
            add(13, lambda: emit_v(13))
            add(13, lambda: emit_qk_chain(False, 1, 1536))
            add(14, lambda: emit_v(14))
            add(14, lambda: emit_qk_chain(True, 1, 0))                  # qt[1] n 0:512
            add(15, lambda: emit_v(15))
            # qt windows ahead of each chunk
            add(20, lambda: emit_qk_chain(True, 0, 512))
            add(36, lambda: emit_qk_chain(True, 1, 512))
            add(52, lambda: emit_qk_chain(True, 0, 1024))
            add(68, lambda: emit_qk_chain(True, 1, 1024))
            add(84, lambda: emit_qk_chain(True, 0, 1536))
            add(100, lambda: emit_qk_chain(True, 1, 1536))
            # output projection for finished chunks; chunk 2's second half is
            # held back for the drain window so the PE has work while the
            # final pair's normalize chain runs
            for c in range(NWIN - 1):
                for msl in range(D // 128):
                    if c == 2 and msl >= 2:
                        continue
                    add(32 * c + 38 + msl, lambda c=c, msl=msl: emit_f(c, msl))

            # ---- attention + output projection ----
            # Software-pipelined by one iteration: the AV matmuls for
            # iteration i-1 are emitted during iteration i, so the exp
            # ACTIVATE of i-1 overlaps the scores+fillers of i and the PE
            # never waits on ScalarE.
            ot_map = {}

            def emit_av(c, p, ms, et):
                if ms == 0:
                    # allocate here (at first use) so the previous pair's
                    # final AV + normalize are already emitted; with bufs=1
                    # the pool dependency then serializes buffer reuse safely
                    ot_map[(c, p)] = [
                        ot_pool.tile([65, NW], F32, tag=f"ot{b_}", name=f"ot{b_}_t")
                        for b_ in range(2)
                    ]
                for b_ in range(2):
                    h = 2 * p + b_
                    nc.tensor.matmul(
                        ot_map[(c, p)][b_],
                        v4[ms][:, h * 65 : (h + 1) * 65],
                        et[:, b_ * NW : (b_ + 1) * NW],
                        start=(ms == 0),
                        stop=(ms == MS - 1),
                    )

            def emit_norm_copies(c, p):
                # stage 1: drain ot to SBUF and launch the denominator
                # shuffle DMAs (sync + gpsimd; the scalar queue must stay
                # clear for the ACTIVATE cadence)
                oraw, rcin = {}, {}
                for b_ in range(2):
                    oraw[b_] = small.tile([65, NW], F32, tag=f"oraw{b_}", name="oraw_t")
                    nc.vector.tensor_copy(out=oraw[b_], in_=ot_map[(c, p)][b_])
                for b_ in range(2):
                    dq = (nc.sync, nc.gpsimd)[b_]
                    rcin[b_] = small.tile([128, NW // 128], F32, tag=f"rcin{b_}", name="rcin_t")
                    dq.dma_start(out=rcin[b_], in_=oraw[b_][64:65, :])
                return oraw, rcin

            def emit_norm_recip(c, p, rcin):
                # stage 2: reciprocal + flatten + broadcast (64 channels —
                # only rows 0:64 feed the normalize multiply)
                rc, rflat, rb = {}, {}, {}
                for b_ in range(2):
                    rc[b_] = small.tile([128, NW // 128], F32, tag=f"rc{b_}", name="rc_t")
                    nc.vector.reciprocal(out=rc[b_], in_=rcin[b_])
                for b_ in range(2):
                    dq = (nc.sync, nc.gpsimd)[b_]
                    rflat[b_] = small.tile([1, NW], F32, tag=f"rflat{b_}", name="rflat_t")
                    dq.dma_start(out=rflat[b_], in_=rc[b_])
                for b_ in range(2):
                    rb[b_] = small.tile([128, NW], F32, tag=f"rb{b_}", name="rb_t")
                    nc.gpsimd.partition_broadcast(rb[b_][0:64, :], rflat[b_])
                return rb

            def emit_norm_mul(c, p, oraw, rb):
                for b_ in range(2):
                    nc.vector.tensor_mul(
                        out=osb_tiles[c][p][b_ * 64 : (b_ + 1) * 64, :],
                        in0=oraw[b_][0:64, :],
                        in1=rb[b_][0:64, :],
                    )

            def emit_norm(c, p):
                oraw, rcin = emit_norm_copies(c, p)
                rb = emit_norm_recip(c, p, rcin)
                emit_norm_mul(c, p, oraw, rb)

            pending = None  # (c, p, ms, et) awaiting its AV matmuls
            iters = [(c, p, ms) for c in range(NWIN) for p in range(NPAIR) for ms in range(MS)]
            for it, (c, p, ms) in enumerate(iters):
                n0 = c * NW
                if p == 0 and ms == 0:
                    osb_tiles[c] = [
                        osb_pool.tile([128, NW], BF16, tag=f"osb{c % 2}_{cs}", name=f"osb{cs}")
                        for cs in range(CH // 128)
                    ]
                st = st_pool.tile([128, 2 * NW], F32, tag="st", name="st_t")
                for b_ in range(2):
                    r0 = b_ * 64
                    nc.tensor.matmul(
                        st[:, b_ * NW : (b_ + 1) * NW],
                        kt[p][r0 : r0 + 64, ms * 128 : (ms + 1) * 128],
                        qt[p][r0 : r0 + 64, n0 : n0 + NW],
                        start=True,
                        stop=True,
                    )
                for f in sched.pop(it, ()):
                    f()
                if pending is not None:
                    pc, pp, pms, pet = pending
                    emit_av(pc, pp, pms, pet)
                    if pms == MS - 1:
                        emit_norm(pc, pp)
                et = et_pool.tile([128, 2 * NW], BF16, tag="et", name="et_t")
                nc.scalar.activation(
                    out=et,
                    in_=st,
                    func=mybir.ActivationFunctionType.Exp,
                    scale=float(1.0 / np.sqrt(DK)),
                )
                pending = (c, p, ms, et)
            # drain the pipeline + the last chunk's output projection; the
            # reserved chunk-2 groups keep the PE warm through the final
            # normalize chain
            # Tail: Tile's per-engine semaphore thresholds are monotonic in
            # emission order, so the interleaving below is load-bearing.
            # The normalize's ot-copies and shuffle DMAs go first; the
            # reserved chunk-2 chains next (their vector-engine casts then
            # sit BETWEEN the copies and the reciprocals in the DVE FIFO,
            # so neither blocks the other — v9 measured the failure mode of
            # putting the reciprocals first); the reciprocal/broadcast
            # stage and the osb muls follow, by which time their DMA
            # latencies have drained behind real PE work.
            pc, pp, pms, pet = pending
            emit_av(pc, pp, pms, pet)
            oraw_t, rcin_t = emit_norm_copies(pc, pp)
            for msl in range(2, D // 128):
                emit_f(NWIN - 2, msl, tail=True)
            rb_t = emit_norm_recip(pc, pp, rcin_t)
            emit_norm_mul(pc, pp, oraw_t, rb_t)
            for msl in range(D // 128):
                emit_f(NWIN - 1, msl, tail=True)
    nc.compile()
    return nc


_NC = None


def _get_nc():
    global _NC
    if _NC is None:
        _NC = _build_bass()
    return _NC


def build_in_maps(inputs):
    x = np.asarray(inputs["x"], dtype=np.float32)
    W_Q = np.asarray(inputs["W_Q"], dtype=np.float32)
    W_K = np.asarray(inputs["W_K"], dtype=np.float32)
    W_V = np.asarray(inputs["W_V"], dtype=np.float32)
    W_O = np.asarray(inputs["W_O"], dtype=np.float32)
    b_Q = np.asarray(inputs["b_Q"], dtype=np.float32)
    b_K = np.asarray(inputs["b_K"], dtype=np.float32)
    b_V = np.asarray(inputs["b_V"], dtype=np.float32)

    in_maps = []
    for c in range(NCORES):
        b, g = divmod(c, GROUPS)
        sl = slice(g * CH, (g + 1) * CH)
        in_maps.append(
            {
                "xT": np.ascontiguousarray(x[b].T.astype(ml_dtypes.bfloat16)),
                "wqT": np.ascontiguousarray(W_Q[sl, :].T.astype(ml_dtypes.bfloat16)),
                "wkT": np.ascontiguousarray(W_K[sl, :].T.astype(ml_dtypes.bfloat16)),
                "wvT": np.ascontiguousarray(W_V[sl, :].T.astype(ml_dtypes.bfloat16)),
                "woT": np.ascontiguousarray(W_O[:, sl].T.astype(ml_dtypes.bfloat16)),
                "bq": np.ascontiguousarray(b_Q[sl]),
                "bk": np.ascontiguousarray(b_K[sl]),
                "bv": np.ascontiguousarray(b_V[sl]),
                "vones": np.ones((128, HPG), dtype=ml_dtypes.bfloat16),
            }
        )
    return in_maps


def kernel(**inputs):
    in_maps = build_in_maps(inputs)
    nc = _get_nc()
    res = run_bass_kernel_spmd(nc, in_maps, core_ids=list(range(NCORES)))

    b_O = np.asarray(inputs["b_O"], dtype=np.float32)
    out = np.zeros((B, N, D), dtype=np.float32)
    for c in range(NCORES):
        b = c // GROUPS
        out[b] += res.results[c]["yT"].T.astype(np.float32)
    out += b_O
    return out
